# revision 2
# baseline (speedup 1.0000x reference)
"""Hamming-distance kernel for Trainium2 (8 NeuronCores, SPMD).

out[n, m] = mean_d(x[n, d] != y[m, d]),  x: (8192, 256), y: (8192, 256),
values are small integers 0..7 stored as float32.

Formulation: categorical equality as a +-1 Hadamard-code GEMM.  Each value
c in {0..7} maps to the 7 non-constant entries of row c of the 8x8
Hadamard matrix: had_j(c) = (-1)^popcount(c & k_j), k_j in {1..7}.  Rows
satisfy <h(a), h(b)> = 8*[a==b] - 1, so with dot[n,m] over K = 7*256 =
1792 features:  eq = (dot + 256)/8  and  out = 1 - eq/256 = 0.875 -
dot/2048.  All code values are +-1 (exact in fp8e4), PSUM accumulates in
fp32 (|dot| <= 1792 << 2^24), and 0.875 - dot*2^-11 is exact binary
arithmetic, so the result is bit-exact.  K = 7 per dim is the provable
minimum embedding for exact categorical equality (vs 8 for one-hot).

Sharding: x rows split across 8 cores (1024 rows each), y replicated.
Each core computes a (1024, 8192) slice of the output.

Device pipeline per core:
  1. DMA x^T shard (256, 1024) and y^T (256, 8192) f32 into fresh SBUF
     slots (host supplies the transposed layout; the feature dim must sit
     on SBUF partitions for the contraction).  Fresh slots => each DMA
     needs at most one sem wait (the DMA ISA has a single wait slot).
  2. Encode +-1 codes k-major: per (m-chunk, d-half): two DVE mods
     (v mod 2, v mod 4), three ACT Sign ops (bit signs s1, s2, s4), four
     DVE products (s3, s5, s6, s7).  y is encoded per m-group into a ring
     of fp8 chunk tiles so encode overlaps the previous group's matmuls.
  3. fp8 DoubleRow GEMM: psum[128, 512] accumulated over 7 k-pairs.
  4. ACT-engine PSUM eviction fused with the affine map 0.875 - dot/2048.
"""

import numpy as np

import concourse.bacc as bacc
import concourse.bass as bass
import concourse.mybir as mybir
import concourse.tile as tile
from concourse.bass_utils import run_bass_kernel_spmd

# Problem dims (hardcoded per contract).
N, M, D, C = 8192, 8192, 256, 8
N_CORES = 8
N_SH = N // N_CORES  # 1024 x-rows per core

P = 128
D_HALVES = D // P  # 2
N_CODES = 7  # +-1 Hadamard code length per dim
KSUB = N_CODES * D_HALVES  # 14 k-subtiles of 128 features -> K = 1792
K_PAIRS = KSUB // 2  # 7 DoubleRow pairs (256 contracted per matmul)
M_CHUNK = 512  # output free-dim tile (one PSUM bank of f32)
M_CHUNKS = M // M_CHUNK  # 16
N_TILES = N_SH // P  # 8
M_GROUP = 4  # m-chunks per psum group (4 banks busy, 8 total)
M_GROUPS = M_CHUNKS // M_GROUP  # 4
MG_COLS = M_GROUP * M_CHUNK  # 2048 m-columns per group

FP8 = mybir.dt.float8e4
F32 = mybir.dt.float32
I32 = mybir.dt.int32
ALU = mybir.AluOpType
ACTF = mybir.ActivationFunctionType


def _encode_half(nc, tmp_pool, dst, h, src, w, biases):
    """Write the 7 +-1 code tiles for d-half h of raw tile src[:, h, :w]
    into dst[:, 2*j + h, :w], j = 0..6.

    Code order j: masks [1, 2, 4, 3, 5, 6, 7]; s_k(v) = (-1)^popcount(v&k).
    s1/s2/s4 from bit signs (ACT Sign), rest are DVE products.
    """
    v = src[:, h, :w]
    b05, b15, b35 = biases

    def slot(j):
        return dst[:, 2 * j + h, :w]

    vi = tmp_pool.tile([P, w], I32, name="enc_vi")
    nc.vector.tensor_copy(vi[:], v)
    t0 = tmp_pool.tile([P, w], I32, name="enc_t0")
    nc.vector.tensor_scalar(
        out=t0[:], in0=vi[:], scalar1=1, scalar2=None, op0=ALU.bitwise_and
    )
    u = tmp_pool.tile([P, w], I32, name="enc_u")
    nc.vector.tensor_scalar(
        out=u[:], in0=vi[:], scalar1=2, scalar2=None, op0=ALU.bitwise_and
    )
    s1, s2, s4 = slot(0), slot(1), slot(2)
    # sign(-t + b): +1 when bit clear, -1 when set
    nc.scalar.activation(s1, t0[:], ACTF.Sign, bias=b05[:], scale=-1.0)
    nc.scalar.activation(s2, u[:], ACTF.Sign, bias=b15[:], scale=-1.0)
    nc.scalar.activation(s4, v, ACTF.Sign, bias=b35[:], scale=-1.0)
    s3, s5, s6, s7 = slot(3), slot(4), slot(5), slot(6)
    nc.vector.tensor_tensor(s3, s1, s2, ALU.mult)
    nc.vector.tensor_tensor(s5, s1, s4, ALU.mult)
    nc.vector.tensor_tensor(s6, s2, s4, ALU.mult)
    nc.vector.tensor_tensor(s7, s3, s4, ALU.mult)


def _build_bass(repeats: int = 1, loop_scope: str = "all"):
    # Bacc (not raw Bass): its compile() legalizes multi-semaphore waits
    # into EventSemaphore instructions (HW allows 1 wait per instruction).
    nc = bacc.Bacc(
        "TRN2", target_bir_lowering=False, debug=False, num_devices=N_CORES
    )

    xt_d = nc.dram_tensor("xt", [D, N_SH], F32, kind="ExternalInput")
    yt_d = nc.dram_tensor("yt", [D, M], F32, kind="ExternalInput")
    # Blocked output layout: block (n, mc) is one contiguous 128x512 f32
    # region, so store DMAs are interval-disjoint (no false WAW chains) and
    # fully contiguous.  The host de-blocks with a transpose+reshape.
    out_d = nc.dram_tensor(
        "out", [N_TILES, M_CHUNKS, P, M_CHUNK], F32, kind="ExternalOutput"
    )

    xt_r = xt_d.rearrange("(h p) n -> p h n", p=P)
    yt_r = yt_d.rearrange("(h p) m -> p h m", p=P)

    with tile.TileContext(nc) as tc:
        mm_only = loop_scope == "mm"
        with (
            tc.tile_pool(name="xe", bufs=1) as xe_pool,
            tc.tile_pool(name="ye", bufs=M_CHUNKS if mm_only else 2 * M_GROUP) as ye_pool,
            tc.tile_pool(name="xraw", bufs=1) as xraw_pool,
            tc.tile_pool(name="yraw", bufs=1 if mm_only else M_GROUPS) as yraw_pool,
            tc.tile_pool(name="tmp", bufs=2 if mm_only else 4) as tmp_pool,
            tc.tile_pool(name="out", bufs=6 if mm_only else 8) as out_pool,
            tc.tile_pool(name="psum", bufs=8, space="PSUM") as psum_pool,
        ):
            # ---- bias constants for ACT Sign ----
            biases = []
            for val in (0.5, 1.5, 3.5):
                b = tmp_pool.tile([P, 1], F32, name=f"bias_{val}", bufs=1)
                nc.vector.memset(b[:], val)
                biases.append(b)

            # ---- raw loads: all into fresh slots ----
            xt_sb = xraw_pool.tile([P, D_HALVES, N_SH], F32)
            nc.sync.dma_start(xt_sb[:], xt_r)
            yraw_tiles = []
            if loop_scope != "mm":
                for mg in range(M_GROUPS):
                    yt_sb = yraw_pool.tile(
                        [P, D_HALVES, MG_COLS], F32, name="yt_sb"
                    )
                    nc.sync.dma_start(
                        yt_sb[:], yt_r[:, :, mg * MG_COLS : (mg + 1) * MG_COLS]
                    )
                    yraw_tiles.append(yt_sb)

            # ---- x codes ----
            xe = xe_pool.tile([P, KSUB, N_SH], FP8)
            for h in range(D_HALVES):
                _encode_half(nc, tmp_pool, xe, h, xt_sb, N_SH, biases)

            pre_encoded = {}
            if loop_scope == "mm":
                # ablation mode: encode everything once, loop only the GEMM
                for mg in range(M_GROUPS):
                    yt_sb = yraw_pool.tile(
                        [P, D_HALVES, MG_COLS], F32, name="yt_sb"
                    )
                    nc.sync.dma_start(
                        yt_sb[:], yt_r[:, :, mg * MG_COLS : (mg + 1) * MG_COLS]
                    )
                    tiles = []
                    for j in range(M_GROUP):
                        ye_mc = ye_pool.tile(
                            [P, KSUB, M_CHUNK], FP8, name="ye_mc"
                        )
                        for h in range(D_HALVES):
                            _encode_half(
                                nc, tmp_pool, ye_mc, h,
                                yt_sb[
                                    :, :, j * M_CHUNK : (j + 1) * M_CHUNK
                                ],
                                M_CHUNK, biases,
                            )
                        tiles.append(ye_mc)
                    pre_encoded[mg] = tiles

            def _one_pass():
                for mg in range(M_GROUPS):
                    # y codes for this m-group (ring; encode of group g+1
                    # overlaps matmuls of group g)
                    if loop_scope == "mm":
                        ye_tiles = pre_encoded[mg]
                    else:
                        ye_tiles = []
                        for j in range(M_GROUP):
                            ye_mc = ye_pool.tile(
                                [P, KSUB, M_CHUNK], FP8, name="ye_mc"
                            )
                            for h in range(D_HALVES):
                                _encode_half(
                                    nc,
                                    tmp_pool,
                                    ye_mc,
                                    h,
                                    yraw_tiles[mg][
                                        :, :, j * M_CHUNK : (j + 1) * M_CHUNK
                                    ],
                                    M_CHUNK,
                                    biases,
                                )
                            ye_tiles.append(ye_mc)

                    for n in range(N_TILES):
                        psum_tiles = [
                            psum_pool.tile([P, M_CHUNK], F32, name="psum")
                            for _ in range(M_GROUP)
                        ]
                        for kp in range(K_PAIRS):
                            lhsT = xe[:, 2 * kp : 2 * kp + 2, n * P : (n + 1) * P]
                            for j in range(M_GROUP):
                                nc.tensor.matmul(
                                    psum_tiles[j][:],
                                    lhsT,
                                    ye_tiles[j][:, 2 * kp : 2 * kp + 2, :],
                                    start=(kp == 0),
                                    stop=(kp == K_PAIRS - 1),
                                    perf_mode=mybir.MatmulPerfMode.DoubleRow,
                                )
                        for j in range(M_GROUP):
                            ot = out_pool.tile([P, M_CHUNK], F32, name="ot")
                            # out = 0.875 - dot/2048  (exact)
                            nc.scalar.activation(
                                ot[:],
                                psum_tiles[j][:],
                                ACTF.Copy,
                                bias=0.875,
                                scale=-1.0 / 2048.0,
                            )
                            mc = mg * M_GROUP + j
                            nc.sync.dma_start(out_d[n, mc], ot[:])

            if repeats == 1:
                _one_pass()
            else:
                # device-side repeat loop, used only for wall-clock timing
                with tc.For_i(0, repeats, 1):
                    _one_pass()
    nc.compile()
    return nc


_NC_CACHE = {}


def _get_nc(repeats: int = 1):
    if repeats not in _NC_CACHE:
        _NC_CACHE[repeats] = _build_bass(repeats)
    return _NC_CACHE[repeats]


def _make_in_maps(x: np.ndarray, y: np.ndarray):
    yt = np.ascontiguousarray(y.T)  # (256, 8192)
    in_maps = []
    for i in range(N_CORES):
        xt_i = np.ascontiguousarray(x[i * N_SH : (i + 1) * N_SH].T)  # (256, 1024)
        in_maps.append({"xt": xt_i, "yt": yt})
    return in_maps


def _deblock(blocked: np.ndarray) -> np.ndarray:
    # (N_TILES, M_CHUNKS, P, M_CHUNK) -> (N_SH, M)
    return np.ascontiguousarray(
        blocked.transpose(0, 2, 1, 3).reshape(N_SH, M)
    )


def kernel(x: np.ndarray, y: np.ndarray, _trace: bool = False):
    x = np.asarray(x, dtype=np.float32)
    y = np.asarray(y, dtype=np.float32)
    assert x.shape == (N, D) and y.shape == (M, D)

    nc = _get_nc(1)
    in_maps = _make_in_maps(x, y)
    res = run_bass_kernel_spmd(
        nc, in_maps, core_ids=list(range(N_CORES)), trace=_trace
    )
    out = np.concatenate([_deblock(r["out"]) for r in res.results], axis=0)
    if _trace:
        return out, res
    return out



# revision 3
# speedup vs baseline: 1.0083x; 1.0083x over previous
"""Hamming-distance kernel for Trainium2 (8 NeuronCores, SPMD) — v3.

out[n, m] = mean_d(x[n, d] != y[m, d]),  x: (8192, 256), y: (8192, 256),
values are small integers 0..7 stored as float32.

Formulation: categorical equality as a +-1 Hadamard-code GEMM (see v2).
eq = (dot + 256)/8, out = 0.875 - dot/2048; exact until the bf16 store.

v3 changes vs v2 (fixes the 44 us of PE idle seen in the v2 trace):
  * m-chunk groups sized [2, 2, 4, 4, 4]: the first matmul group is
    gated on a single 1.8 MB ye chunk, cutting the DMA lead-in.
  * PE warm-up: a burst of dummy matmuls with no input deps runs during
    the DMA lead-in so HAM is at K=8/8 when the real stream starts.
  * Output is evicted into per-(group, n) staging tiles and stored as
    one contiguous [128, G*512] bf16 block -> 2-4 KB per-partition DMA
    descriptors (v2's 1 KB blocks capped stores at ~87 GB/s, exactly the
    steady-state demand, and backed up the PSUM pipeline).

Sharding: x rows split across 8 cores (1024 rows each), y replicated.
"""

import ml_dtypes
import numpy as np

import concourse.bacc as bacc
import concourse.mybir as mybir
import concourse.tile as tile
from concourse.bass_utils import run_bass_kernel_spmd

# Problem dims (hardcoded per contract).
N, M, D, C = 8192, 8192, 256, 8
N_CORES = 8
N_SH = N // N_CORES  # 1024 x-rows per core

P = 128
D_HALVES = D // P  # 2
N_CODES = 7  # +-1 Hadamard code length per dim
KSUB = N_CODES * D_HALVES  # 14 k-subtiles of 128 features -> K = 1792
K_PAIRS = KSUB // 2  # 7 DoubleRow pairs (256 contracted per matmul)
M_CHUNK = 512  # output free-dim tile (one PSUM bank of f32)
M_CHUNKS = M // M_CHUNK  # 16
N_TILES = N_SH // P  # 8
Y_CHUNK = 1024  # m-cols per ye DMA chunk
Y_CHUNKS = M // Y_CHUNK  # 8
# m-chunk group sizes: first groups small so matmuls start after 1 chunk
GROUP_SIZES = [2, 2, 4, 4, 4]
WARMUP_MMS = 12

FP8 = mybir.dt.float8e4
F32 = mybir.dt.float32
BF16 = mybir.dt.bfloat16
ACTF = mybir.ActivationFunctionType
NP_FP8 = ml_dtypes.float8_e4m3fn

# code LUT: LUT[v, j] = (-1)^popcount(v & mask_j), masks [1,2,4,3,5,6,7]
_MASKS = np.array([1, 2, 4, 3, 5, 6, 7], dtype=np.uint8)
_POP = np.array([bin(i).count("1") for i in range(8)], dtype=np.uint8)
_LUT = np.where(
    _POP[np.arange(8, dtype=np.uint8)[:, None] & _MASKS[None, :]] % 2 == 0,
    np.float32(1.0),
    np.float32(-1.0),
).astype(NP_FP8)  # (8, 7)


def _build_bass():
    nc = bacc.Bacc(
        "TRN2", target_bir_lowering=False, debug=False, num_devices=N_CORES
    )

    # chunk-major DRAM layouts: each DMA chunk is fully contiguous.
    xe_d = nc.dram_tensor("xe", [N_TILES, P, KSUB, P], FP8, kind="ExternalInput")
    ye_d = nc.dram_tensor(
        "ye", [Y_CHUNKS, P, KSUB, Y_CHUNK], FP8, kind="ExternalInput"
    )
    # Output blocks per (group, n): contiguous [P, G*512] bf16 per block.
    out2_d = nc.dram_tensor(
        "out2", [2, N_TILES, P, 2 * M_CHUNK], BF16, kind="ExternalOutput"
    )
    out4_d = nc.dram_tensor(
        "out4", [3, N_TILES, P, 4 * M_CHUNK], BF16, kind="ExternalOutput"
    )

    with tile.TileContext(nc) as tc:
        with (
            tc.tile_pool(name="xe", bufs=1) as xe_pool,
            tc.tile_pool(name="ye", bufs=1) as ye_pool,
            tc.tile_pool(name="warm", bufs=1) as warm_pool,
            tc.tile_pool(name="out", bufs=8) as out_pool,
            tc.tile_pool(name="psum", bufs=8, space="PSUM") as psum_pool,
        ):
            # ---- PE warm-up: dummy matmuls, no input deps ----
            wsrc = warm_pool.tile([P, 2, M_CHUNK], FP8, name="wsrc")
            nc.vector.memset(wsrc[:], 0.0)
            wps = psum_pool.tile([P, M_CHUNK], F32, name="psum")
            for _ in range(WARMUP_MMS):
                nc.tensor.matmul(
                    wps[:],
                    wsrc[:, :, :P],
                    wsrc[:],
                    start=True,
                    stop=True,
                    perf_mode=mybir.MatmulPerfMode.DoubleRow,
                )

            # ---- loads (issue order = arrival priority) ----
            xe = xe_pool.tile([P, N_TILES, KSUB, P], FP8)
            ye = ye_pool.tile([P, Y_CHUNKS, KSUB, Y_CHUNK], FP8)
            nc.sync.dma_start(xe[:, 0], xe_d[0])
            nc.sync.dma_start(ye[:, 0], ye_d[0])
            for n in range(1, N_TILES):
                nc.sync.dma_start(xe[:, n], xe_d[n])
            for c in range(1, Y_CHUNKS):
                nc.sync.dma_start(ye[:, c], ye_d[c])

            # ---- GEMM + evict + store ----
            mc0 = 0
            for mg, G in enumerate(GROUP_SIZES):
                blk = mg if G == 2 else mg - 2
                out_d = out2_d if G == 2 else out4_d
                for n in range(N_TILES):
                    psum_tiles = [
                        psum_pool.tile([P, M_CHUNK], F32, name="psum")
                        for _ in range(G)
                    ]
                    for kp in range(K_PAIRS):
                        lhsT = xe[:, n, 2 * kp : 2 * kp + 2, :]
                        for j in range(G):
                            mc = mc0 + j
                            c, hh = divmod(mc, 2)
                            nc.tensor.matmul(
                                psum_tiles[j][:],
                                lhsT,
                                ye[
                                    :,
                                    c,
                                    2 * kp : 2 * kp + 2,
                                    hh * M_CHUNK : (hh + 1) * M_CHUNK,
                                ],
                                start=(kp == 0),
                                stop=(kp == K_PAIRS - 1),
                                perf_mode=mybir.MatmulPerfMode.DoubleRow,
                            )
                    ot = out_pool.tile([P, G * M_CHUNK], BF16, name="ot")
                    for j in range(G):
                        # out = 0.875 - dot/2048  (exact before bf16 store)
                        nc.scalar.activation(
                            ot[:, j * M_CHUNK : (j + 1) * M_CHUNK],
                            psum_tiles[j][:],
                            ACTF.Copy,
                            bias=0.875,
                            scale=-1.0 / 2048.0,
                        )
                    nc.sync.dma_start(out_d[blk, n], ot[:])
                mc0 += G
    nc.compile()
    return nc


_NC_CACHE = {}


def _get_nc():
    if "nc" not in _NC_CACHE:
        _NC_CACHE["nc"] = _build_bass()
    return _NC_CACHE["nc"]


def _encode(vals: np.ndarray, n_tiles: int, tile_cols: int) -> np.ndarray:
    """(rows, 256) int values -> (n_tiles, 128, 14, tile_cols) fp8 codes.

    Element [t, p, 2j+h, q] = code_j(vals[t*tile_cols + q, h*128 + p]).
    """
    rows = vals.shape[0]
    assert rows == n_tiles * tile_cols
    codes = _LUT[vals.astype(np.uint8)]  # (rows, 256, 7) fp8
    codes = codes.reshape(n_tiles, tile_cols, D_HALVES, P, N_CODES)
    # [t, q, h, p, j] -> [t, p, j, h, q]
    codes = codes.transpose(0, 3, 4, 2, 1)
    return np.ascontiguousarray(codes).reshape(n_tiles, P, KSUB, tile_cols)


def _make_in_maps(x: np.ndarray, y: np.ndarray):
    ye = _encode(y, Y_CHUNKS, Y_CHUNK)
    in_maps = []
    for i in range(N_CORES):
        xe = _encode(x[i * N_SH : (i + 1) * N_SH], N_TILES, P)
        in_maps.append({"xe": xe, "ye": ye})
    return in_maps


def _assemble(r) -> np.ndarray:
    # out2: (2, 8, 128, 1024), out4: (3, 8, 128, 2048) bf16 -> (N_SH, M) f32
    o2 = np.asarray(r["out2"])
    o4 = np.asarray(r["out4"])
    full = np.empty((N_SH, M), dtype=np.float32)
    mc0 = 0
    for mg, G in enumerate(GROUP_SIZES):
        w = G * M_CHUNK
        src = o2[mg] if G == 2 else o4[mg - 2]  # (8, 128, w)
        full[:, mc0 * M_CHUNK : mc0 * M_CHUNK + w] = src.reshape(
            N_SH, w
        ).astype(np.float32)
        mc0 += G
    return full


def kernel(x: np.ndarray, y: np.ndarray, _trace: bool = False):
    x = np.asarray(x, dtype=np.float32)
    y = np.asarray(y, dtype=np.float32)
    assert x.shape == (N, D) and y.shape == (M, D)

    nc = _get_nc()
    in_maps = _make_in_maps(x, y)
    res = run_bass_kernel_spmd(
        nc, in_maps, core_ids=list(range(N_CORES)), trace=_trace
    )
    out = np.concatenate([_assemble(r) for r in res.results], axis=0)
    if _trace:
        return out, res
    return out
